# revision 7
# baseline (speedup 1.0000x reference)
"""Trainium2 Bass kernel for nn_NewsClassifier (GCN news classifier, 8 NeuronCores).

Self-contained: takes FULL inputs as numpy arrays, shards internally (SPMD over 8
cores), returns (preds [64,2] fp32, loss scalar fp32) matching reference().

Design:
- Nodes of each branch (repost/comment, 100000 each) are renumbered: sorted by
  in-degree, round-robin over 816 = 8 cores x 102 tiles bins -> per-tile in-edge
  counts are balanced (repost <=512 = 4 blocks of 128, comment <=384 = 3 blocks).
  Padded to 13056 rows/core.
- Phase A per branch: rx = X @ W_text per shard (bf16), AllGather -> rx_full.
- Conv layer 1 per dst tile: self-loop via diag(w_self) matmul + indirect-DMA
  row gathers (1 offset/partition) of rx_full[src] + weighted one-hot S matmul
  scatter into PSUM; then x W1, relu -> a1 tile (never stored!).
- Layer 2 + mean-pool collapse (pooling is linear):
  pool(Â (a1 W2)) = ((Pool∘Â) @ a1) @ W2. Per-tile matmul with host-built dense
  PoolÂ columns accumulates [64,256] partial; AllReduce 128KB; @W2 on device.
- Head (replicated on all cores): reps=[content,repost,comment,video,image]@...,
  log-softmax NLL loss on device. Core 0's outputs are returned.

Host preprocessing touches ONLY integer graph structure (edges/batch vectors)
and scalar weights derived from it (degrees, 1/cnt); all feature-tensor math
runs on device.
"""
import numpy as np
import ml_dtypes

N_CORES = 8
PN = 13056            # padded nodes per core
NT = PN // 128        # 102 tiles per core
NTOT = N_CORES * PN   # 104448
NBINS = N_CORES * NT  # 816
D, H, B, G = 768, 256, 64, 512
NN = 100000           # nodes per branch
CAP_R, BPT_R = 512, 4    # repost: edge slots per tile, blocks per tile
CAP_C, BPT_C = 384, 3    # comment
NBLK_R, NBLK_C = NT * BPT_R, NT * BPT_C   # 408, 306 blocks per core
BF = ml_dtypes.bfloat16


def _shard_branch(edge_index, cap, bpt):
    """Renumber nodes (degree-balanced round-robin) and slot edges per core.

    Returns dict with:
      gpos [NN] int64: node -> global padded position (core*PN + tile*128 + slot)
      rows_of_core: list of [PN] int64 (original node id or -1 for pad)
      dinv [NN] f32
      per-core meta: idx [128, NT*bpt] i32, dm/wv [128, NT*bpt] f32,
                     wself [128, NT] f32
    """
    src = edge_index[0].astype(np.int64)
    dst = edge_index[1].astype(np.int64)
    indeg = np.bincount(dst, minlength=NN)
    deg = (indeg + 1).astype(np.float32)
    dinv = (1.0 / np.sqrt(deg)).astype(np.float32)

    order = np.argsort(-indeg, kind="stable")
    rank = np.empty(NN, dtype=np.int64)
    rank[order] = np.arange(NN)
    bins = rank % NBINS
    core_of = bins % N_CORES
    tile_of = bins // N_CORES
    slot_of = rank // NBINS
    assert slot_of.max() < 128
    gpos = core_of * PN + tile_of * 128 + slot_of

    rows_of_core = []
    for c in range(N_CORES):
        rows = np.full(PN, -1, dtype=np.int64)
        m = core_of == c
        rows[tile_of[m] * 128 + slot_of[m]] = np.nonzero(m)[0]
        rows_of_core.append(rows)

    w_e = dinv[src] * dinv[dst]
    ecore = core_of[dst]
    metas = []
    for c in range(N_CORES):
        m = ecore == c
        es, ed, ew = src[m], dst[m], w_e[m]
        et = tile_of[ed]
        o = np.argsort(et, kind="stable")
        es, ed, ew, et = es[o], ed[o], ew[o], et[o]
        cnt = np.bincount(et, minlength=NT)
        assert cnt.max() <= cap, (cnt.max(), cap)
        starts = np.zeros(NT, dtype=np.int64)
        starts[1:] = np.cumsum(cnt)[:-1]
        j = np.arange(es.size) - np.repeat(starts, cnt)
        slot = et * cap + j
        blk = slot // 128
        p = slot % 128
        nblk = NT * bpt
        idx = np.zeros((128, nblk), dtype=np.int32)
        dm = np.zeros((128, nblk), dtype=np.float32)
        wv = np.zeros((128, nblk), dtype=np.float32)
        idx[p, blk] = gpos[es]
        dm[p, blk] = slot_of[ed]
        wv[p, blk] = ew
        wself = np.zeros((128, NT), dtype=np.float32)
        rows = rows_of_core[c]
        mrow = rows >= 0
        ww = np.zeros(PN, dtype=np.float32)
        ww[mrow] = dinv[rows[mrow]] ** 2
        wself[:, :] = ww.reshape(NT, 128).T
        metas.append(dict(idx=idx, dm=dm, wv=wv, wself=wself))
    return dict(gpos=gpos, rows_of_core=rows_of_core, dinv=dinv,
                src=src, dst=dst, w_e=w_e, metas=metas)


def _pool_matrix(sh, node_pool_w, b_of_node):
    """Dense (Pool∘Â) [B, NTOT] f32: PA[b, gpos(s)] = sum over edges (s->d) of
    node_pool_w[d]*w_e [b=b_of_node[d]] + self-loop terms."""
    PA = np.zeros((B, NTOT), dtype=np.float64)
    src, dst, w_e = sh["src"], sh["dst"], sh["w_e"]
    np.add.at(PA, (b_of_node[dst], sh["gpos"][src]),
              (node_pool_w[dst] * w_e).astype(np.float64))
    allnodes = np.arange(NN)
    np.add.at(PA, (b_of_node[allnodes], sh["gpos"][allnodes]),
              (node_pool_w * sh["dinv"] ** 2).astype(np.float64))
    return PA.astype(np.float32)


def preprocess(inputs):
    """-> (shards dict for both branches, per-core in_maps extras)."""
    out = {}
    shr = _shard_branch(inputs["repost_edge_index"], CAP_R, BPT_R)
    shc = _shard_branch(inputs["comment_edge_index"], CAP_C, BPT_C)

    rb = inputs["repost_batch"].astype(np.int64)
    cnt_r = np.bincount(rb, minlength=B).astype(np.float32)
    pw_r = 1.0 / np.maximum(cnt_r, 1.0)
    PA_r = _pool_matrix(shr, pw_r[rb], rb)

    cb = inputs["comment_batch"].astype(np.int64)
    cgb = inputs["comment_graph_batch"].astype(np.int64)
    cnt1 = np.bincount(cb, minlength=G).astype(np.float32)
    cnt2 = np.bincount(cgb, minlength=B).astype(np.float32)
    node_w = (1.0 / np.maximum(cnt1, 1.0))[cb] * (1.0 / np.maximum(cnt2, 1.0))[cgb[cb]]
    PA_c = _pool_matrix(shc, node_w.astype(np.float32), cgb[cb])

    out["shr"], out["shc"] = shr, shc
    out["PA_r"], out["PA_c"] = PA_r, PA_c

    lab = inputs["label"].astype(np.int64)
    onehot = np.zeros((B, 2), dtype=np.float32)
    onehot[np.arange(B), lab] = 1.0
    out["onehot"] = onehot
    return out


def _core_xshard(X, rows):
    xs = np.zeros((PN, D), dtype=np.float32)
    m = rows >= 0
    xs[m] = X[rows[m]]
    return xs


def _pa_sbuf_layout(PA, c):
    """PA [B, NTOT] -> per-core SBUF layout [128, NT*B] bf16 (lhsT tiles)."""
    cols = PA[:, c * PN:(c + 1) * PN]              # [64, PN]
    t = cols.T.reshape(NT, 128, B)                  # [NT,128,B]
    return np.ascontiguousarray(t.transpose(1, 0, 2).reshape(128, NT * B)).astype(BF)


def build_in_maps(inputs, pre):
    iota = np.tile(np.arange(128, dtype=np.float32)[None, :], (128, 1))
    ones64 = np.ones((B, 1), dtype=np.float32)
    b1 = inputs["b1"].astype(np.float32).reshape(2, 128).T.copy()   # [128,2]
    b2 = inputs["b2"].astype(np.float32).reshape(2, 1)
    common = {
        "w_text": inputs["W_text"].astype(np.float32),
        "w_video": inputs["W_video"].astype(np.float32),
        "w_image": inputs["W_image"].astype(np.float32),
        "wr1": inputs["Wr1"].astype(np.float32),
        "wr2": inputs["Wr2"].astype(np.float32),
        "wc1": inputs["Wc1"].astype(np.float32),
        "wc2": inputs["Wc2"].astype(np.float32),
        "w1h": inputs["W1"].astype(np.float32),
        "w2h": inputs["W2"].astype(np.float32),
        "b1": b1, "b2": b2,
        "content": inputs["content"].astype(np.float32),
        "video": inputs["video"].astype(np.float32),
        "image": inputs["image"].astype(np.float32),
        "onehot": pre["onehot"], "iota": iota, "ones64": ones64,
    }
    in_maps = []
    for c in range(N_CORES):
        mr, mc = pre["shr"]["metas"][c], pre["shc"]["metas"][c]
        im = dict(common)
        im["xr"] = _core_xshard(inputs["repost_x"], pre["shr"]["rows_of_core"][c])
        im["xc"] = _core_xshard(inputs["comment_x"], pre["shc"]["rows_of_core"][c])
        im["idxr"], im["dmr"], im["wvr"] = mr["idx"], mr["dm"], mr["wv"]
        im["idxc"], im["dmc"], im["wvc"] = mc["idx"], mc["dm"], mc["wv"]
        im["wselfr"], im["wselfc"] = mr["wself"], mc["wself"]
        im["par"] = _pa_sbuf_layout(pre["PA_r"], c)
        im["pac"] = _pa_sbuf_layout(pre["PA_c"], c)
        in_maps.append(im)
    return in_maps


# ---------------------------------------------------------------------------
# numpy host simulation of the sharded algorithm (for validating preprocessing)
# ---------------------------------------------------------------------------

def host_sim(inputs, pre, in_maps):
    Wt = inputs["W_text"]

    def branch(xkey, idxk, dmk, wvk, wsk, pak, W1, cap, bpt):
        rx_shards = []
        for c in range(N_CORES):
            rx_shards.append(in_maps[c][xkey] @ Wt)
        rx_full = np.concatenate(rx_shards, axis=0)          # [NTOT, 256]
        pooled = np.zeros((B, H), dtype=np.float64)
        for c in range(N_CORES):
            im = in_maps[c]
            idx, dm, wv, ws = im[idxk], im[dmk], im[wvk], im[wsk]
            a1_tiles = []
            for t in range(NT):
                z = ws[:, t][:, None] * rx_shards[c][t * 128:(t + 1) * 128]  # self
                for b in range(t * bpt, (t + 1) * bpt):
                    g = rx_full[idx[:, b]]                   # [128, 256]
                    S = (np.arange(128)[None, :] == dm[:, b][:, None]) * wv[:, b][:, None]
                    z = z + S.T @ g
                a1 = np.maximum(z @ W1, 0.0)
                a1_tiles.append(a1)
            a1s = np.concatenate(a1_tiles, axis=0)           # [PN, 256]
            pa = im[pak].astype(np.float32).reshape(128, NT, B).transpose(1, 0, 2).reshape(PN, B)
            pooled += pa.T.astype(np.float64) @ a1s
        return pooled.astype(np.float32)

    pr = branch("xr", "idxr", "dmr", "wvr", "wselfr", "par", inputs["Wr1"], CAP_R, BPT_R)
    pc = branch("xc", "idxc", "dmc", "wvc", "wselfc", "pac", inputs["Wc1"], CAP_C, BPT_C)
    repost_reps = pr @ inputs["Wr2"]
    comment_reps = pc @ inputs["Wc2"]
    content_reps = inputs["content"] @ Wt
    video_reps = inputs["video"] @ inputs["W_video"]
    image_reps = inputs["image"] @ inputs["W_image"]
    reps = np.concatenate([content_reps, repost_reps, comment_reps,
                           video_reps, image_reps], axis=-1)
    h = np.maximum(reps @ inputs["W1"] + inputs["b1"], 0.0)
    preds = h @ inputs["W2"] + inputs["b2"]
    m = preds.max(axis=-1, keepdims=True)
    lse = m + np.log(np.exp(preds - m).sum(axis=-1, keepdims=True))
    logp = preds - lse
    loss = -np.mean(logp[np.arange(B), inputs["label"].astype(np.int64)])
    return preds, np.float32(loss)


# ---------------------------------------------------------------------------
# Bass program
# ---------------------------------------------------------------------------

_CACHE = {}


def build_program():
    if "nc" in _CACHE:
        return _CACHE["nc"]
    import concourse.bacc as bacc
    import concourse.bass as bass
    import concourse.mybir as mybir
    import concourse.tile as tile
    from concourse.masks import make_identity

    BF16 = mybir.dt.bfloat16
    F32 = mybir.dt.float32
    I32 = mybir.dt.int32
    AF = mybir.ActivationFunctionType
    ALU = mybir.AluOpType
    AX = mybir.AxisListType

    nc = bacc.Bacc("TRN2", target_bir_lowering=False, debug=False,
                   enable_asserts=False, num_devices=N_CORES)

    def din(name, shape, dt=F32):
        return nc.dram_tensor(name, shape, dt, kind="ExternalInput")

    xr = din("xr", [PN, D]); xc = din("xc", [PN, D])
    w_text = din("w_text", [D, H]); w_video = din("w_video", [D, H])
    w_image = din("w_image", [D, H])
    wr1 = din("wr1", [H, H]); wr2 = din("wr2", [H, H])
    wc1 = din("wc1", [H, H]); wc2 = din("wc2", [H, H])
    w1h = din("w1h", [5 * H, H]); w2h = din("w2h", [H, 2])
    b1 = din("b1", [128, 2]); b2 = din("b2", [2, 1])
    content = din("content", [B, D]); video = din("video", [B, D])
    image = din("image", [B, D])
    onehot = din("onehot", [B, 2]); iota = din("iota", [128, 128])
    ones64 = din("ones64", [B, 1])
    idxr = din("idxr", [128, NBLK_R], I32); idxc = din("idxc", [128, NBLK_C], I32)
    dmr = din("dmr", [128, NBLK_R]); dmc = din("dmc", [128, NBLK_C])
    wvr = din("wvr", [128, NBLK_R]); wvc = din("wvc", [128, NBLK_C])
    wselfr = din("wselfr", [128, NT]); wselfc = din("wselfc", [128, NT])
    par = din("par", [128, NT * B], BF16); pac = din("pac", [128, NT * B], BF16)

    preds_out = nc.dram_tensor("preds", [B, 2], F32, kind="ExternalOutput")
    loss_out = nc.dram_tensor("loss", [1, 1], F32, kind="ExternalOutput")

    import os
    STAGE = int(os.environ.get("KERNEL_STAGE", "3"))
    with tile.TileContext(nc) as tc:
        with (
            tc.tile_pool(name="dram", bufs=1, space="DRAM") as dpool,
            tc.tile_pool(name="const", bufs=1) as cpool,
            tc.tile_pool(name="stage", bufs=2) as stpool,
            tc.tile_pool(name="xin", bufs=3) as xpool,
            tc.tile_pool(name="xbf", bufs=2) as xbfpool,
            tc.tile_pool(name="tp", bufs=2, space="PSUM") as tppool,
            tc.tile_pool(name="mm", bufs=2, space="PSUM") as mmpool,
            tc.tile_pool(name="zp", bufs=2, space="PSUM") as zpool,
            tc.tile_pool(name="pacc", bufs=1, space="PSUM") as paccpool,
            tc.tile_pool(name="xts", bufs=2) as xtspool,
            tc.tile_pool(name="rxs", bufs=3) as rxspool,
            tc.tile_pool(name="gat", bufs=6) as gpool,
            tc.tile_pool(name="sblk", bufs=4) as spool,
            tc.tile_pool(name="work", bufs=4) as wpool,
        ):
            agin_r = dpool.tile([PN, H], BF16, name="agin_r")
            agin_c = dpool.tile([PN, H], BF16, name="agin_c")
            rxf_r = dpool.tile([NTOT, H], BF16, addr_space="Shared", name="rxf_r")
            rxf_c = dpool.tile([NTOT, H], BF16, addr_space="Shared", name="rxf_c")
            arin = dpool.tile([128, H], F32, name="arin")
            arout = dpool.tile([128, H], F32, addr_space="Shared", name="arout")
            # ---- constants & weights ----
            ident_bf = cpool.tile([128, 128], BF16)
            make_identity(nc, ident_bf[:])
            ident_f32 = cpool.tile([128, 128], F32)
            make_identity(nc, ident_f32[:])
            iota_sb = cpool.tile([128, 128], F32)
            nc.sync.dma_start(out=iota_sb[:], in_=iota[:])

            def load_w_bf(dram, rows, cols, name):
                kch = rows // 128
                f = stpool.tile([128, kch * cols], F32, name=f"{name}_f32", tag="stage")
                bfv = cpool.tile([128, kch * cols], BF16, name=f"{name}_bf")
                for k in range(kch):
                    nc.sync.dma_start(out=f[:, k * cols:(k + 1) * cols],
                                      in_=dram[k * 128:(k + 1) * 128, :])
                nc.vector.tensor_copy(out=bfv[:], in_=f[:])
                return bfv

            wt_bf = load_w_bf(w_text, D, H, "wt")
            wv_bf = load_w_bf(w_video, D, H, "wv")
            wi_bf = load_w_bf(w_image, D, H, "wi")
            wr1_bf = load_w_bf(wr1, H, H, "wr1")
            wc1_bf = load_w_bf(wc1, H, H, "wc1")
            wr2_bf = load_w_bf(wr2, H, H, "wr2")
            wc2_bf = load_w_bf(wc2, H, H, "wc2")
            w1h_bf = load_w_bf(w1h, 5 * H, H, "w1h")
            w2h_bf = load_w_bf(w2h, H, 2, "w2h")
            b1_sb = cpool.tile([128, 2], F32)
            nc.sync.dma_start(out=b1_sb[:], in_=b1[:])
            b2_sb = cpool.tile([2, 1], F32)
            nc.sync.dma_start(out=b2_sb[:], in_=b2[:])
            onehot_sb = cpool.tile([B, 2], F32)
            nc.sync.dma_start(out=onehot_sb[:], in_=onehot[:])
            ones64_sb = cpool.tile([B, 1], F32)
            nc.sync.dma_start(out=ones64_sb[:], in_=ones64[:])

            # meta (resident)
            idxr_sb = cpool.tile([128, NBLK_R], I32)
            nc.sync.dma_start(out=idxr_sb[:], in_=idxr[:])
            dmr_sb = cpool.tile([128, NBLK_R], F32)
            nc.sync.dma_start(out=dmr_sb[:], in_=dmr[:])
            wvr_sb = cpool.tile([128, NBLK_R], F32)
            nc.sync.dma_start(out=wvr_sb[:], in_=wvr[:])
            idxc_sb = cpool.tile([128, NBLK_C], I32)
            nc.sync.dma_start(out=idxc_sb[:], in_=idxc[:])
            dmc_sb = cpool.tile([128, NBLK_C], F32)
            nc.sync.dma_start(out=dmc_sb[:], in_=dmc[:])
            wvc_sb = cpool.tile([128, NBLK_C], F32)
            nc.sync.dma_start(out=wvc_sb[:], in_=wvc[:])
            wsr_sb = cpool.tile([128, NT], F32)
            nc.sync.dma_start(out=wsr_sb[:], in_=wselfr[:])
            wsc_sb = cpool.tile([128, NT], F32)
            nc.sync.dma_start(out=wsc_sb[:], in_=wselfc[:])
            par_sb = cpool.tile([128, NT * B], BF16)
            nc.sync.dma_start(out=par_sb[:], in_=par[:])
            pac_sb = cpool.tile([128, NT * B], BF16)
            nc.sync.dma_start(out=pac_sb[:], in_=pac[:])

            KD = D // 128

            def phase_a(xdram, agin):
                for t in range(NT):
                    xt = xpool.tile([128, D], F32, name="xt")
                    nc.sync.dma_start(out=xt[:], in_=xdram[t * 128:(t + 1) * 128, :])
                    xb = xbfpool.tile([128, D], BF16, name="xb")
                    nc.vector.tensor_copy(out=xb[:], in_=xt[:])
                    xT = xtspool.tile([128, KD, 128], BF16, name="xT")
                    for k in range(KD):
                        tp = tppool.tile([128, 128], BF16, space="PSUM", name="tpa", tag="tpa")
                        nc.tensor.transpose(out=tp[:], in_=xb[:, k * 128:(k + 1) * 128],
                                            identity=ident_bf[:])
                        nc.vector.tensor_copy(out=xT[:, k, :], in_=tp[:])
                    rp = mmpool.tile([128, H], F32, space="PSUM", name="rpa", tag="rpa")
                    for k in range(KD):
                        nc.tensor.matmul(out=rp[:], lhsT=xT[:, k, :],
                                         rhs=wt_bf[:, k * H:(k + 1) * H],
                                         start=(k == 0), stop=(k == KD - 1))
                    ro = rxspool.tile([128, H], BF16, name="ro")
                    nc.vector.tensor_copy(out=ro[:], in_=rp[:])
                    nc.sync.dma_start(out=agin[t * 128:(t + 1) * 128, :], in_=ro[:])

            def allgather(agin, rxf):
                nc.gpsimd.collective_compute(
                    "AllGather", mybir.AluOpType.bypass,
                    ins=[agin[:]], outs=[rxf[:]],
                    replica_groups=[list(range(N_CORES))])

            def layer1(agin, rxf, idx_sb, dm_sb, wv_sb, ws_sb, w1_bf, pa_sb,
                       bpt, pool_psum):
                for t in range(NT):
                    rxo = gpool.tile([128, H], BF16, name="rxo", tag="gat")
                    nc.sync.dma_start(out=rxo[:], in_=agin[t * 128:(t + 1) * 128, :])
                    diag = spool.tile([128, 128], BF16, name="diag", tag="sblk")
                    nc.vector.tensor_scalar_mul(out=diag[:], in0=ident_bf[:],
                                                scalar1=ws_sb[:, t:t + 1])
                    zp = zpool.tile([128, H], F32, space="PSUM", name="zp")
                    nc.tensor.matmul(out=zp[:], lhsT=diag[:], rhs=rxo[:],
                                     start=True, stop=False)
                    for b in range(t * bpt, (t + 1) * bpt):
                        g = gpool.tile([128, H], BF16, name="g", tag="gat")
                        nc.gpsimd.indirect_dma_start(
                            out=g[:], out_offset=None, in_=rxf[:],
                            in_offset=bass.IndirectOffsetOnAxis(
                                ap=idx_sb[:, b:b + 1], axis=0))
                        S = spool.tile([128, 128], BF16, name="S", tag="sblk")
                        nc.vector.tensor_scalar(
                            out=S[:], in0=iota_sb[:],
                            scalar1=dm_sb[:, b:b + 1], scalar2=wv_sb[:, b:b + 1],
                            op0=ALU.is_equal, op1=ALU.mult)
                        nc.tensor.matmul(out=zp[:], lhsT=S[:], rhs=g[:],
                                         start=False, stop=(b == (t + 1) * bpt - 1))
                    z1 = wpool.tile([128, H], BF16, name="z1", tag="work")
                    nc.vector.tensor_copy(out=z1[:], in_=zp[:])
                    z1T = wpool.tile([128, 2, 128], BF16, name="z1T", tag="work")
                    for k in range(2):
                        tp = tppool.tile([128, 128], BF16, space="PSUM", name="tpa", tag="tpa")
                        nc.tensor.transpose(out=tp[:], in_=z1[:, k * 128:(k + 1) * 128],
                                            identity=ident_bf[:])
                        nc.vector.tensor_copy(out=z1T[:, k, :], in_=tp[:])
                    t1 = mmpool.tile([128, H], F32, space="PSUM", name="rpa", tag="rpa")
                    for k in range(2):
                        nc.tensor.matmul(out=t1[:], lhsT=z1T[:, k, :],
                                         rhs=w1_bf[:, k * H:(k + 1) * H],
                                         start=(k == 0), stop=(k == 1))
                    a1 = wpool.tile([128, H], BF16, name="a1", tag="work")
                    nc.scalar.activation(out=a1[:], in_=t1[:], func=AF.Relu)
                    nc.tensor.matmul(out=pool_psum[:], lhsT=pa_sb[:, t * B:(t + 1) * B],
                                     rhs=a1[:], start=(t == 0), stop=(t == NT - 1))

            # ---------------- emission order ----------------
            phase_a(xr, agin_r)
            allgather(agin_r, rxf_r)
            phase_a(xc, agin_c)
            allgather(agin_c, rxf_c)

            arsb = cpool.tile([128, H], F32)
            pool_r = paccpool.tile([B, H], F32, space="PSUM", name="poolacc")
            layer1(agin_r, rxf_r, idxr_sb, dmr_sb, wvr_sb, wsr_sb, wr1_bf,
                   par_sb, BPT_R, pool_r)
            nc.vector.tensor_copy(out=arsb[0:B, :], in_=pool_r[:])
            pool_c = paccpool.tile([B, H], F32, space="PSUM", name="poolacc")
            layer1(agin_c, rxf_c, idxc_sb, dmc_sb, wvc_sb, wsc_sb, wc1_bf,
                   pac_sb, BPT_C, pool_c)
            poolc_sb = wpool.tile([B, H], F32, name="poolc_sb", tag="work2")
            nc.vector.tensor_copy(out=poolc_sb[:], in_=pool_c[:])
            nc.sync.dma_start(out=arsb[B:2 * B, :], in_=poolc_sb[:])

            nc.sync.dma_start(out=arin[:], in_=arsb[:])
            nc.gpsimd.collective_compute(
                "AllReduce", mybir.AluOpType.add,
                ins=[arin[:]], outs=[arout[:]],
                replica_groups=[list(range(N_CORES))])
            ars2_r = cpool.tile([B, H], F32)
            nc.sync.dma_start(out=ars2_r[:], in_=arout[0:B, :])
            ars2_c = cpool.tile([B, H], F32)
            nc.sync.dma_start(out=ars2_c[:], in_=arout[B:2 * B, :])

            # ---------------- head (replicated, B=64) ----------------
            # transposed reps pieces: list of [128, 64] bf16 chunks (10 = 5*2)
            reps_T = []

            def add_modality(x_dram, w_bf):
                xt = wpool.tile([B, D], F32, name="mx", tag="work2")
                nc.sync.dma_start(out=xt[:], in_=x_dram[:])
                xb = wpool.tile([B, D], BF16, name="mxb", tag="work2")
                nc.vector.tensor_copy(out=xb[:], in_=xt[:])
                xT = wpool.tile([128, KD, B], BF16, name="mxT", tag="work3")
                for k in range(KD):
                    tp = tppool.tile([128, B], BF16, space="PSUM", name="tph", tag="tpa")
                    nc.tensor.transpose(out=tp[:], in_=xb[:, k * 128:(k + 1) * 128],
                                        identity=ident_bf[:B, :B])
                    nc.vector.tensor_copy(out=xT[:, k, :], in_=tp[:])
                for m in range(2):
                    mp = mmpool.tile([128, B], F32, space="PSUM", name="rph", tag="rpa")
                    for k in range(KD):
                        nc.tensor.matmul(
                            out=mp[:],
                            lhsT=wt_slice(w_bf, k, m),
                            rhs=xT[:, k, :], start=(k == 0), stop=(k == KD - 1))
                    rt = cpool.tile([128, B], BF16, name=f"repsT{len(reps_T)}")
                    nc.vector.tensor_copy(out=rt[:], in_=mp[:])
                    reps_T.append(rt)

            def wt_slice(w_bf, k, m):
                # w_bf layout [128, kch*H]; chunk (k,m) = [128, 128]
                return w_bf[:, k * H + m * 128: k * H + (m + 1) * 128]

            def add_pooled(ars2, w2_bf):
                # pooled raw [64, 256]; result (pooled@W2)^T
                pT = wpool.tile([128, 2, B], BF16, name="pT", tag="work3")
                for k in range(2):
                    tp = tppool.tile([128, B], F32, space="PSUM", name="tph2", tag="tpa")
                    nc.tensor.transpose(out=tp[:], in_=ars2[:, k * 128:(k + 1) * 128],
                                        identity=ident_f32[:B, :B])
                    nc.vector.tensor_copy(out=pT[:, k, :], in_=tp[:])
                for m in range(2):
                    mp = mmpool.tile([128, B], F32, space="PSUM", name="rph", tag="rpa")
                    for k in range(2):
                        nc.tensor.matmul(
                            out=mp[:], lhsT=wt_slice(w2_bf, k, m),
                            rhs=pT[:, k, :], start=(k == 0), stop=(k == 1))
                    rt = cpool.tile([128, B], BF16, name=f"repsT{len(reps_T)}")
                    nc.vector.tensor_copy(out=rt[:], in_=mp[:])
                    reps_T.append(rt)

            add_modality(content, wt_bf)
            add_pooled(ars2_r, wr2_bf)
            add_pooled(ars2_c, wc2_bf)
            add_modality(video, wv_bf)
            add_modality(image, wi_bf)

            # h1T = relu(W1h^T @ repsT + b1): 2 m-chunks of [128, 64]
            h1T = wpool.tile([128, 2, B], BF16, name="h1T", tag="work3")
            for m in range(2):
                hp = mmpool.tile([128, B], F32, space="PSUM", name="rph", tag="rpa")
                for k in range(10):
                    nc.tensor.matmul(out=hp[:], lhsT=wt_slice(w1h_bf, k, m),
                                     rhs=reps_T[k][:], start=(k == 0), stop=(k == 9))
                nc.scalar.activation(out=h1T[:, m, :], in_=hp[:], func=AF.Relu,
                                     bias=b1_sb[:, m:m + 1])
            # predsT [2, 64] = W2h^T @ h1T + b2
            pp = mmpool.tile([2, B], F32, space="PSUM", name="rph", tag="rpa")
            for k in range(2):
                nc.tensor.matmul(out=pp[:], lhsT=w2h_bf[:, k * 2:(k + 1) * 2],
                                 rhs=h1T[:, k, :], start=(k == 0), stop=(k == 1))
            predsT = wpool.tile([2, B], F32, name="predsT", tag="work2")
            nc.vector.tensor_scalar_add(out=predsT[:], in0=pp[:], scalar1=b2_sb[:])
            # transpose -> preds [64, 2] f32
            ppt = tppool.tile([B, 2], F32, space="PSUM", name="tph3", tag="tpa")
            nc.tensor.transpose(out=ppt[:], in_=predsT[:], identity=ident_f32[:2, :2])
            preds = cpool.tile([B, 2], F32)
            nc.vector.tensor_copy(out=preds[:], in_=ppt[:])
            nc.sync.dma_start(out=preds_out[:], in_=preds[:])

            # loss = -mean(logp[label]) = mean(lse - p_label)
            rmax = wpool.tile([B, 1], F32, name="rmax", tag="red")
            nc.vector.reduce_max(out=rmax[:], in_=preds[:], axis=AX.X)
            xm = wpool.tile([B, 2], F32, name="xm", tag="red2")
            nc.vector.tensor_scalar(out=xm[:], in0=preds[:], scalar1=rmax[:],
                                    scalar2=None, op0=ALU.subtract)
            ex = wpool.tile([B, 2], F32, name="ex", tag="red2")
            nc.scalar.activation(out=ex[:], in_=xm[:], func=AF.Exp)
            sm = wpool.tile([B, 1], F32, name="sm", tag="red")
            nc.vector.reduce_sum(out=sm[:], in_=ex[:], axis=AX.X)
            ls = wpool.tile([B, 1], F32, name="ls", tag="red")
            nc.scalar.activation(out=ls[:], in_=sm[:], func=AF.Ln)
            lse = wpool.tile([B, 1], F32, name="lse", tag="red")
            nc.vector.tensor_tensor(out=lse[:], in0=rmax[:], in1=ls[:], op=ALU.add)
            pl = wpool.tile([B, 2], F32, name="pl", tag="red2")
            nc.vector.tensor_tensor(out=pl[:], in0=preds[:], in1=onehot_sb[:],
                                    op=ALU.mult)
            plr = wpool.tile([B, 1], F32, name="plr", tag="red")
            nc.vector.reduce_sum(out=plr[:], in_=pl[:], axis=AX.X)
            nll = wpool.tile([B, 1], F32, name="nll", tag="red")
            nc.vector.tensor_tensor(out=nll[:], in0=lse[:], in1=plr[:],
                                    op=ALU.subtract)
            lp = mmpool.tile([1, 1], F32, space="PSUM", name="rph", tag="rpa")
            # need bf16?? keep f32 matmul: lhsT nll [64,1] f32, rhs ones [64,1] f32
            nc.tensor.matmul(out=lp[:], lhsT=nll[:], rhs=ones64_sb[:],
                             start=True, stop=True)
            lossv = cpool.tile([1, 1], F32)
            nc.scalar.activation(out=lossv[:], in_=lp[:], func=AF.Copy,
                                 scale=1.0 / B)
            nc.sync.dma_start(out=loss_out[:], in_=lossv[:])

    nc.compile()
    _CACHE["nc"] = nc
    return nc


def kernel(**inputs):
    inputs = {k: np.asarray(v) for k, v in inputs.items()}
    pre = preprocess(inputs)
    in_maps = build_in_maps(inputs, pre)
    nc = build_program()
    from concourse.bass_utils import run_bass_kernel_spmd
    res = run_bass_kernel_spmd(nc, in_maps, list(range(N_CORES)))
    preds = np.asarray(res.results[0]["preds"], dtype=np.float32)
    loss = np.float32(np.asarray(res.results[0]["loss"]).reshape(()))
    return preds, loss
